# revision 1
# baseline (speedup 1.0000x reference)
"""Multi-head embedding lookup on 8 NeuronCores.

Sharding: head h -> core h. Each core owns one head's 100000x64 f32 table
shard and produces rows for all B*L = 65536 tokens of that head.

Per-core algorithm (int16-indexed Ant SWDGE ucode ops, <=1024 descriptors
per instruction -- the SWDGE ring limit measured on HW):
  - Table shard split into TQ=4 quartiles of 25000 rows (int16-addressable).
  - Tokens of each quartile, in token order, balanced-split into NCH_q
    chunks of <=1024.  Chunk j's token positions cluster around
    NTOK*j/NCH_q, so a static 32768-row output window per chunk keeps
    scatter positions int16-addressable (bases are computed from the
    actual inputs and asserted).
  - Per chunk: dma_gather (queue 0) table window rows -> SBUF wrap layout;
    dma_scatter_add (queue 1) SBUF rows -> out[token position] (+= onto the
    pre-zeroed output).
  - Chunks are padded to 1024 with -1 (skipped by the ucode); true counts
    ride in an int32 side tensor -> Pool registers.
"""

import contextlib
import os
import sys

if "/opt/trn_rl_repo" not in sys.path:
    sys.path.insert(0, "/opt/trn_rl_repo")

_SP = os.environ.get("KSP", "1") == "1"      # single_packet for gather/scatter
_SORT = os.environ.get("KSORT", "1") == "1"  # sort chunk entries by gather idx
_GQ = int(os.environ.get("KGQ", "1"))        # gather queues (round-robin)
_NQ = int(os.environ.get("KNQ", "2"))        # total swdge queues

import numpy as np

import concourse.bacc as bacc
import concourse.bass as bass
import concourse.mybir as mybir
from concourse.bass_utils import run_bass_kernel_spmd
from concourse.library_config import mlp

H = 8          # heads == cores
NH = 100000    # table rows per head
D = 64         # embedding dim
B, L = 16, 4096
NTOK = B * L   # tokens per head = 65536
TQ, QSZ = 4, 25000   # table quartiles
CAP = int(os.environ.get("KCAP", "1024"))  # max descriptors per SWDGE instruction
WIN = 32768          # scatter window rows (int16 reach)
NSLOT = int(os.environ.get("KNSLOT", "6"))  # SBUF data ring slots

_CACHE: dict = {}


def _core_quartiles(idx: np.ndarray):
    """idx: [NTOK] int64 -> per quartile (positions int64, local gather idx int64)."""
    q_of = idx // QSZ
    out = []
    for q in range(TQ):
        pos = np.nonzero(q_of == q)[0]
        out.append((pos, idx[pos] - q * QSZ))
    return out


def _plan(all_quart):
    """all_quart: [H][TQ](pos, gloc).  Returns static plan:
    chunks = list of (q, base) in issue order; nch[q]."""
    nch = []
    for q in range(TQ):
        nch.append(max(-(-len(all_quart[h][q][0]) // CAP) for h in range(H)))
    chunks = []
    for q in range(TQ):
        for j in range(nch[q]):
            base = NTOK
            lo_needed = 0
            for h in range(H):
                pos = all_quart[h][q][0]
                part = np.array_split(pos, nch[q])[j]
                assert len(part) > 0
                base = min(base, int(part[0]))
                lo_needed = max(lo_needed, int(part[-1]))
            assert lo_needed - base < WIN, (q, j, base, lo_needed)
            base = min(base, NTOK - WIN)  # keep window inside the tensor
            chunks.append((q, base))
    return chunks, nch


def _wrap16(vals: np.ndarray, pad: int) -> np.ndarray:
    a = np.full(pad, -1, dtype=np.int16)
    a[: len(vals)] = vals
    return a.reshape(pad // 16, 16).T  # [16, pad//16]


def _prep_core(quart, chunks, nch):
    ncols = len(chunks) * (CAP // 16)
    gidx_all = np.full((16, ncols), -1, dtype=np.int16)
    sidx_all = np.full((16, ncols), -1, dtype=np.int16)
    counts = np.zeros((1, len(chunks)), dtype=np.int32)
    # chunk order in `chunks` is q-major, j-minor — mirror it
    ci = 0
    for q in range(TQ):
        pos, gloc = quart[q]
        parts = np.array_split(np.arange(len(pos)), nch[q])
        for j in range(nch[q]):
            sel = parts[j]
            base = chunks[ci][1]
            g = gloc[sel]
            s = pos[sel] - base
            if _SORT:
                order = np.argsort(g, kind="stable")
                g = g[order]
                s = s[order]
            assert s.min() >= 0 and s.max() < WIN
            w = CAP // 16
            gidx_all[:, ci * w : (ci + 1) * w] = _wrap16(g.astype(np.int16), CAP)
            sidx_all[:, ci * w : (ci + 1) * w] = _wrap16(s.astype(np.int16), CAP)
            counts[0, ci] = len(sel)
            ci += 1
    gidx_all = np.ascontiguousarray(np.tile(gidx_all, (8, 1)))
    sidx_all = np.ascontiguousarray(np.tile(sidx_all, (8, 1)))
    return gidx_all, sidx_all, counts


def _build_nc(chunks, reps: int = 1, mode: str = "full") -> bass.Bass:
    NCH = len(chunks)
    ncols = NCH * (CAP // 16)
    nc = bacc.Bacc("TRN2", num_swdge_queues=_NQ)
    table = nc.dram_tensor("table", [NH, D], mybir.dt.float32, kind="ExternalInput")
    gidx = nc.dram_tensor("gidx", [128, ncols], mybir.dt.int16, kind="ExternalInput")
    sidx = nc.dram_tensor("sidx", [128, ncols], mybir.dt.int16, kind="ExternalInput")
    counts = nc.dram_tensor("counts", [1, NCH], mybir.dt.int32, kind="ExternalInput")
    out = nc.dram_tensor("out", [NTOK, D], mybir.dt.float32, kind="ExternalOutput")

    with contextlib.ExitStack() as ctx:
        gidx_t = ctx.enter_context(nc.sbuf_tensor("gidx_t", [128, ncols], mybir.dt.int16))
        sidx_t = ctx.enter_context(nc.sbuf_tensor("sidx_t", [128, ncols], mybir.dt.int16))
        counts_t = ctx.enter_context(nc.sbuf_tensor("counts_t", [1, NCH], mybir.dt.int32))
        data = [
            ctx.enter_context(
                nc.sbuf_tensor(f"data{s}", [128, CAP // 128, D], mybir.dt.float32)
            )
            for s in range(NSLOT)
        ]
        in_sems = [ctx.enter_context(nc.semaphore(f"in_sem{i}")) for i in range(3)]
        g_sems = [ctx.enter_context(nc.semaphore(f"g_sem{s}")) for s in range(NSLOT)]
        st_sems = [ctx.enter_context(nc.semaphore(f"st_sem{s}")) for s in range(NSLOT)]
        block = ctx.enter_context(nc.Block())

        @block.sync
        def _(sp):
            sp.dma_start(out=gidx_t[:], in_=gidx[:]).then_inc(in_sems[0], 16)
            sp.dma_start(out=sidx_t[:], in_=sidx[:]).then_inc(in_sems[1], 16)
            sp.dma_start(out=counts_t[:], in_=counts[:]).then_inc(in_sems[2], 16)

        @block.gpsimd
        def _(gp):
            gp.load_library(mlp)
            for i in range(3):
                gp.wait_ge(in_sems[i], 16)

            with contextlib.ExitStack() as rctx:
                cnt = [
                    rctx.enter_context(gp.register(f"cnt{s}")) for s in range(NSLOT)
                ]
                w = CAP // 16

                TOT = NCH * reps

                def issue_gather(t):
                    s, r = t % NSLOT, t // NSLOT
                    k = t % NCH
                    q = chunks[k][0]
                    if r > 0:
                        gp.wait_ge(st_sems[s], 16 * r)
                    gp.reg_load(cnt[s], counts_t[0:1, k : k + 1])
                    gp.dma_gather(
                        out_ap=data[s][:],
                        in_ap=table[q * QSZ : (q + 1) * QSZ, :],
                        idxs_ap=gidx_t[:, k * w : (k + 1) * w],
                        num_idxs=CAP,
                        num_idxs_reg=cnt[s],
                        elem_size=D,
                        queue_num=t % _GQ,
                        single_packet=_SP,
                    ).then_inc(g_sems[s], 16)

                def issue_scatter(t):
                    s, r = t % NSLOT, t // NSLOT
                    k = t % NCH
                    base = chunks[k][1]
                    gp.wait_ge(g_sems[s], 16 * (r + 1))
                    gp.dma_scatter_add(
                        out_ap=out[base : base + WIN, :],
                        in_ap=data[s][:],
                        idxs_ap=sidx_t[:, k * w : (k + 1) * w],
                        num_idxs=CAP,
                        num_idxs_reg=cnt[s],
                        elem_size=D,
                        queue_num=_NQ - 1,
                        single_packet=_SP,
                    ).then_inc(st_sems[s], 16)

                if mode == "full":
                    for t in range(TOT):
                        issue_gather(t)
                        if t >= 1:
                            issue_scatter(t - 1)
                    issue_scatter(TOT - 1)
                    for s in range(NSLOT):
                        gp.wait_ge(st_sems[s], 16 * len(range(s, TOT, NSLOT)))
                elif mode == "gather":
                    # timing probe: gathers only, no slot backpressure needed
                    for t in range(TOT):
                        s, k = t % NSLOT, t % NCH
                        q = chunks[k][0]
                        gp.reg_load(cnt[s], counts_t[0:1, k : k + 1])
                        gp.dma_gather(
                            out_ap=data[s][:],
                            in_ap=table[q * QSZ : (q + 1) * QSZ, :],
                            idxs_ap=gidx_t[:, k * w : (k + 1) * w],
                            num_idxs=CAP,
                            num_idxs_reg=cnt[s],
                            elem_size=D,
                            queue_num=0,
                        ).then_inc(g_sems[s], 16)
                    for s in range(NSLOT):
                        gp.wait_ge(g_sems[s], 16 * len(range(s, TOT, NSLOT)))
                elif mode == "scatter":
                    # timing probe: scatter garbage SBUF repeatedly
                    for t in range(TOT):
                        s, k = t % NSLOT, t % NCH
                        base = chunks[k][1]
                        gp.reg_load(cnt[s], counts_t[0:1, k : k + 1])
                        gp.dma_scatter_add(
                            out_ap=out[base : base + WIN, :],
                            in_ap=data[s][:],
                            idxs_ap=sidx_t[:, k * w : (k + 1) * w],
                            num_idxs=CAP,
                            num_idxs_reg=cnt[s],
                            elem_size=D,
                            queue_num=1,
                        ).then_inc(st_sems[s], 16)
                    for s in range(NSLOT):
                        gp.wait_ge(st_sems[s], 16 * len(range(s, TOT, NSLOT)))
                elif mode == "regload":
                    # timing probe: just the reg_loads
                    for t in range(TOT):
                        s, k = t % NSLOT, t % NCH
                        gp.reg_load(cnt[s], counts_t[0:1, k : k + 1])
                    gp.wait_ge(in_sems[0], 16)
                else:
                    raise ValueError(mode)

    nc.compile()
    return nc


def _get_nc(chunks):
    key = ("nc", tuple(chunks))
    if key not in _CACHE:
        _CACHE[key] = _build_nc(chunks)
    return _CACHE[key]


def kernel(input_ids: np.ndarray, table: np.ndarray, **_run_kw) -> np.ndarray:
    input_ids = np.asarray(input_ids)
    table = np.asarray(table, dtype=np.float32)

    all_quart = [
        _core_quartiles(input_ids[:, :, h].reshape(-1).astype(np.int64))
        for h in range(H)
    ]
    chunks, nch = _plan(all_quart)
    nc = _get_nc(chunks)

    in_maps = []
    for h in range(H):
        gidx_all, sidx_all, counts = _prep_core(all_quart[h], chunks, nch)
        tab_h = np.ascontiguousarray(table[h * NH : (h + 1) * NH])
        in_maps.append(
            {"table": tab_h, "gidx": gidx_all, "sidx": sidx_all, "counts": counts}
        )

    res = run_bass_kernel_spmd(nc, in_maps, list(range(H)), **_run_kw)
    outs = [
        np.asarray(res.results[h]["out"]).reshape(B, L, D) for h in range(H)
    ]
    full = np.stack(outs, axis=2)  # [B, L, H, D]
    if _run_kw:
        _CACHE["last_results"] = res
    return full



# revision 17
# speedup vs baseline: 1.3783x; 1.3783x over previous
"""Multi-head embedding lookup on 8 NeuronCores.

Sharding: head h -> core h. Each core owns one head's 100000x64 f32 table
shard and produces rows for all B*L = 65536 tokens of that head.

Per-core algorithm (int16-indexed Ant SWDGE ucode ops, <=1024 descriptors
per instruction -- the SWDGE ring limit measured on HW):
  - Table shard split into TQ=25 windows of 4000 rows (int16-addressable;
    narrow windows give the sorted gather streams ~4.6-row average gaps,
    i.e. HBM row-buffer locality).
  - Tokens of each window, in token order, balanced-split into NCH_q
    chunks of <=1024.  Chunk j's token positions cluster around
    NTOK*j/NCH_q, so a static 32768-row output window per chunk keeps
    scatter positions int16-addressable (bases are computed from the
    actual inputs and asserted).
  - Per chunk: dma_gather (queues 0-1 round-robin) table window rows ->
    SBUF wrap layout; dma_scatter_add (queues 2-3) SBUF rows ->
    out[token position] (+= onto the pre-zeroed output).  4 SWDGE queues
    roughly double descriptor drain throughput vs 2.
  - LAYOUT="lane": each chunk's (gather, scatter) index pairs are sorted
    by gather idx and laid out as 16 contiguous sorted runs, one per
    descriptor lane (desc j <- idxs[j%16, j//16]), so each lane's HBM
    read stream is ascending.  ORDER="base": chunks are issued in token-
    base order, interleaving table windows across the in-flight set
    (spreads HBM banks) and keeping scatter writes region-concentrated.
  - Chunks are padded to 1024 with -1 (skipped by the ucode); true counts
    ride in an int32 side tensor -> Pool registers (num_idxs_reg MUST
    equal the exact valid count -- the ucode emits exactly that many
    descriptors; a larger value crashes the device).
  - NSLOT=12 SBUF data slots keep ~3 instructions in flight per queue.
"""

import contextlib
import os
import sys

if "/opt/trn_rl_repo" not in sys.path:
    sys.path.insert(0, "/opt/trn_rl_repo")

_SP = os.environ.get("KSP", "1") == "1"      # single_packet for gather/scatter
_SORT = os.environ.get("KSORT", "1") == "1"  # sort chunk entries by gather idx
_GQ = int(os.environ.get("KGQ", "2"))        # gather queues (round-robin)
_NQ = int(os.environ.get("KNQ", "4"))        # total swdge queues
_SQ = int(os.environ.get("KSQ", "2"))        # scatter queues

import numpy as np

import concourse.bacc as bacc
import concourse.bass as bass
import concourse.mybir as mybir
from concourse.bass_utils import run_bass_kernel_spmd
from concourse.library_config import mlp

H = 8          # heads == cores
NH = 100000    # table rows per head
D = 64         # embedding dim
B, L = 16, 4096
NTOK = B * L   # tokens per head = 65536
_KTQ = int(os.environ.get("KTQ", "25"))
TQ, QSZ = _KTQ, 100000 // _KTQ   # table windows (gather idx int16 reach)
CAP = int(os.environ.get("KCAP", "1024"))  # max descriptors per SWDGE instruction
WIN = 32768          # scatter window rows (int16 reach)
NSLOT = int(os.environ.get("KNSLOT", "12"))  # SBUF data ring slots

_CACHE: dict = {}


PAD = "neg"      # "neg": -1 suffix + true counts; "zero": pad to CAP with
                 # zero-table-row gathers and +0 scatters (uniform count)
LAYOUT = os.environ.get("KLAYOUT", "lane")  # "wrap" | "lane" (sorted runs/lane)
ORDER = os.environ.get("KORDER", "base")    # "qmajor" | "base" (token-base major)


def configure(
    tq=None, cap=None, nslot=None, sort=None, pad=None, layout=None, order=None
):
    """Adjust plan-shape globals (probe helper; defaults stay for grading)."""
    global TQ, QSZ, CAP, NSLOT, _SORT, PAD, LAYOUT, ORDER
    if tq is not None:
        assert NH % tq == 0
        TQ, QSZ = tq, NH // tq
    if cap is not None:
        CAP = cap
    if nslot is not None:
        NSLOT = nslot
    if sort is not None:
        _SORT = bool(sort)
    if pad is not None:
        assert pad in ("neg", "zero")
        PAD = pad
    if layout is not None:
        assert layout in ("wrap", "lane")
        LAYOUT = layout
    if order is not None:
        assert order in ("qmajor", "base")
        ORDER = order
    _CACHE.clear()


def _table_rows():
    """Device table rows: +1 zero row per window when PAD=='zero'."""
    return TQ * (QSZ + 1) if PAD == "zero" else NH


def _win_base(q):
    """Device row of window q's first table row."""
    return q * (QSZ + 1) if PAD == "zero" else q * QSZ


def _win_rows():
    """Gather in_ap rows per window (zero row included when PAD=='zero')."""
    return QSZ + 1 if PAD == "zero" else QSZ


def _core_table(tab_h: np.ndarray) -> np.ndarray:
    """Per-core device table: zero row appended per window when PAD=='zero'."""
    if PAD != "zero":
        return np.ascontiguousarray(tab_h)
    t = np.zeros((TQ * (QSZ + 1), D), dtype=np.float32)
    for q in range(TQ):
        t[q * (QSZ + 1) : q * (QSZ + 1) + QSZ] = tab_h[q * QSZ : (q + 1) * QSZ]
    return t


def _core_quartiles(idx: np.ndarray):
    """idx: [NTOK] int64 -> per quartile (positions int64, local gather idx int64)."""
    q_of = idx // QSZ
    out = []
    for q in range(TQ):
        pos = np.nonzero(q_of == q)[0]
        out.append((pos, idx[pos] - q * QSZ))
    return out


def _plan(all_quart):
    """all_quart: [H][TQ](pos, gloc).  Returns static plan:
    chunks = list of (q, j, base) in issue order; nch[q]."""
    nch = []
    for q in range(TQ):
        nch.append(max(-(-len(all_quart[h][q][0]) // CAP) for h in range(H)))
    chunks = []
    for q in range(TQ):
        for j in range(nch[q]):
            base = NTOK
            lo_needed = 0
            for h in range(H):
                pos = all_quart[h][q][0]
                part = np.array_split(pos, nch[q])[j]
                assert len(part) > 0
                base = min(base, int(part[0]))
                lo_needed = max(lo_needed, int(part[-1]))
            assert lo_needed - base < WIN, (q, j, base, lo_needed)
            base = min(base, NTOK - WIN)  # keep window inside the tensor
            chunks.append((q, j, base))
    if ORDER == "base":
        chunks.sort(key=lambda c: (c[2], c[0]))
    return chunks, nch


def _wrap16(vals: np.ndarray, pad: int) -> np.ndarray:
    a = np.full(pad, -1, dtype=np.int16)
    a[: len(vals)] = vals
    return a.reshape(pad // 16, 16).T  # [16, pad//16]


def _chunk_arrays(g: np.ndarray, s: np.ndarray):
    """Lay one chunk's (gather idx, scatter idx) pairs into [16, CAP//16]
    per-lane arrays honoring PAD/LAYOUT. Returns (ga, sa, count)."""
    c = len(g)
    w = CAP // 16
    if PAD == "zero":
        count = CAP
        gpad, spad = QSZ, 0  # gather the window's zero row, +0 to out[base]
    else:
        count = c
        gpad, spad = -1, -1
    ga = np.full((16, w), gpad, dtype=np.int16)
    sa = np.full((16, w), spad, dtype=np.int16)
    if LAYOUT == "wrap":
        if PAD == "zero":
            gf = np.full(CAP, gpad, dtype=np.int16)
            sf = np.full(CAP, spad, dtype=np.int16)
            gf[:c] = g
            sf[:c] = s
            ga = gf.reshape(w, 16).T.copy()
            sa = sf.reshape(w, 16).T.copy()
        else:
            ga = _wrap16(g.astype(np.int16), CAP)
            sa = _wrap16(s.astype(np.int16), CAP)
    else:  # lane: sorted runs per lane; valid counts balanced (wrap prefix)
        r = c % 16
        start = 0
        for l in range(16):
            n_l = c // 16 + (1 if l < r else 0)
            ga[l, :n_l] = g[start : start + n_l]
            sa[l, :n_l] = s[start : start + n_l]
            start += n_l
    return ga, sa, count


def _prep_core(quart, chunks, nch):
    ncols = len(chunks) * (CAP // 16)
    w = CAP // 16
    gidx_all = np.empty((16, ncols), dtype=np.int16)
    sidx_all = np.empty((16, ncols), dtype=np.int16)
    counts = np.zeros((1, len(chunks) + 1), dtype=np.int32)
    counts[0, -1] = CAP  # shared count cell (valid when PAD=='zero')
    parts_by_q = {
        q: np.array_split(np.arange(len(quart[q][0])), nch[q]) for q in range(TQ)
    }
    for ci, (q, j, base) in enumerate(chunks):
        pos, gloc = quart[q]
        sel = parts_by_q[q][j]
        g = gloc[sel]
        s = pos[sel] - base
        if _SORT:
            order = np.argsort(g, kind="stable")
            g = g[order]
            s = s[order]
        assert s.min() >= 0 and s.max() < WIN
        ga, sa, count = _chunk_arrays(g.astype(np.int16), s.astype(np.int16))
        gidx_all[:, ci * w : (ci + 1) * w] = ga
        sidx_all[:, ci * w : (ci + 1) * w] = sa
        counts[0, ci] = count
    gidx_all = np.ascontiguousarray(np.tile(gidx_all, (8, 1)))
    sidx_all = np.ascontiguousarray(np.tile(sidx_all, (8, 1)))
    return gidx_all, sidx_all, counts


def prep_inputs(input_ids: np.ndarray, table: np.ndarray):
    """Full host prep: returns (chunks, in_maps) for the 8 cores."""
    input_ids = np.asarray(input_ids)
    table = np.asarray(table, dtype=np.float32)
    all_quart = [
        _core_quartiles(input_ids[:, :, h].reshape(-1).astype(np.int64))
        for h in range(H)
    ]
    chunks, nch = _plan(all_quart)
    in_maps = []
    for h in range(H):
        gidx_all, sidx_all, counts = _prep_core(all_quart[h], chunks, nch)
        tab_h = _core_table(table[h * NH : (h + 1) * NH])
        in_maps.append(
            {"table": tab_h, "gidx": gidx_all, "sidx": sidx_all, "counts": counts}
        )
    return chunks, in_maps


def _build_nc(
    chunks,
    reps: int = 1,
    mode: str = "full",
    nq: int | None = None,
    gq: int | None = None,
    sq: int | None = None,
    sp: bool | None = None,
    const_cap: bool = False,
    soff: int = 0,
    gqs: list | None = None,
    sqs: list | None = None,
) -> bass.Bass:
    """nq: total swdge queues; gq: gather queues (0..gq-1);
    sq: scatter queues (nq-sq..nq-1); const_cap: use one shared CAP
    count register instead of per-chunk reg_loads."""
    nq = _NQ if nq is None else nq
    gq = _GQ if gq is None else gq
    sq = _SQ if sq is None else sq
    sp = _SP if sp is None else sp
    if gqs is None:
        gqs = [t % gq for t in range(gq)]
    if sqs is None:
        sqs = [nq - sq + t for t in range(sq)]
    NCH = len(chunks)
    ncols = NCH * (CAP // 16)
    nc = bacc.Bacc(
        "TRN2",
        num_swdge_queues=nq,
        dynamic_dma_scratch_size=max(16384, 16 * CAP),
    )
    table = nc.dram_tensor(
        "table", [_table_rows(), D], mybir.dt.float32, kind="ExternalInput"
    )
    gidx = nc.dram_tensor("gidx", [128, ncols], mybir.dt.int16, kind="ExternalInput")
    sidx = nc.dram_tensor("sidx", [128, ncols], mybir.dt.int16, kind="ExternalInput")
    counts = nc.dram_tensor(
        "counts", [1, NCH + 1], mybir.dt.int32, kind="ExternalInput"
    )
    out = nc.dram_tensor("out", [NTOK, D], mybir.dt.float32, kind="ExternalOutput")

    with contextlib.ExitStack() as ctx:
        gidx_t = ctx.enter_context(nc.sbuf_tensor("gidx_t", [128, ncols], mybir.dt.int16))
        sidx_t = ctx.enter_context(nc.sbuf_tensor("sidx_t", [128, ncols], mybir.dt.int16))
        counts_t = ctx.enter_context(
            nc.sbuf_tensor("counts_t", [1, NCH + 1], mybir.dt.int32)
        )
        data = [
            ctx.enter_context(
                nc.sbuf_tensor(f"data{s}", [128, CAP // 128, D], mybir.dt.float32)
            )
            for s in range(NSLOT)
        ]
        in_sems = [ctx.enter_context(nc.semaphore(f"in_sem{i}")) for i in range(3)]
        g_sems = [ctx.enter_context(nc.semaphore(f"g_sem{s}")) for s in range(NSLOT)]
        st_sems = [ctx.enter_context(nc.semaphore(f"st_sem{s}")) for s in range(NSLOT)]
        block = ctx.enter_context(nc.Block())

        @block.sync
        def _(sp):
            sp.dma_start(out=gidx_t[:], in_=gidx[:]).then_inc(in_sems[0], 16)
            sp.dma_start(out=sidx_t[:], in_=sidx[:]).then_inc(in_sems[1], 16)
            sp.dma_start(out=counts_t[:], in_=counts[:]).then_inc(in_sems[2], 16)

        @block.gpsimd
        def _(gp):
            gp.load_library(mlp)
            for i in range(3):
                gp.wait_ge(in_sems[i], 16)

            with contextlib.ExitStack() as rctx:
                cnt = [
                    rctx.enter_context(gp.register(f"cnt{s}")) for s in range(NSLOT)
                ]
                if const_cap:
                    capreg = rctx.enter_context(gp.register("capreg"))
                    gp.reg_load(capreg, counts_t[0:1, NCH : NCH + 1])
                w = CAP // 16

                TOT = NCH * reps

                def issue_gather(t):
                    s, r = t % NSLOT, t // NSLOT
                    k = t % NCH
                    q = chunks[k][0]
                    if r > 0:
                        gp.wait_ge(st_sems[s], 16 * r)
                    if not const_cap:
                        gp.reg_load(cnt[s], counts_t[0:1, k : k + 1])
                    gp.dma_gather(
                        out_ap=data[s][:],
                        in_ap=table[_win_base(q) : _win_base(q) + _win_rows(), :],
                        idxs_ap=gidx_t[:, k * w : (k + 1) * w],
                        num_idxs=CAP,
                        num_idxs_reg=capreg if const_cap else cnt[s],
                        elem_size=D,
                        queue_num=gqs[t % len(gqs)],
                        single_packet=sp,
                    ).then_inc(g_sems[s], 16)

                def issue_scatter(t):
                    s, r = t % NSLOT, t // NSLOT
                    k = t % NCH
                    base = chunks[k][2]
                    gp.wait_ge(g_sems[s], 16 * (r + 1))
                    gp.dma_scatter_add(
                        out_ap=out[base : base + WIN, :],
                        in_ap=data[s][:],
                        idxs_ap=sidx_t[:, k * w : (k + 1) * w],
                        num_idxs=CAP,
                        num_idxs_reg=capreg if const_cap else cnt[s],
                        elem_size=D,
                        queue_num=sqs[(t + soff) % len(sqs)],
                        single_packet=sp,
                    ).then_inc(st_sems[s], 16)

                if mode == "full":
                    for t in range(TOT):
                        issue_gather(t)
                        if t >= 1:
                            issue_scatter(t - 1)
                    issue_scatter(TOT - 1)
                    for s in range(NSLOT):
                        gp.wait_ge(st_sems[s], 16 * len(range(s, TOT, NSLOT)))
                elif mode == "gather":
                    # timing probe: gathers only, no slot backpressure needed
                    for t in range(TOT):
                        s, k = t % NSLOT, t % NCH
                        q = chunks[k][0]
                        gp.reg_load(cnt[s], counts_t[0:1, k : k + 1])
                        gp.dma_gather(
                            out_ap=data[s][:],
                            in_ap=table[_win_base(q) : _win_base(q) + _win_rows(), :],
                            idxs_ap=gidx_t[:, k * w : (k + 1) * w],
                            num_idxs=CAP,
                            num_idxs_reg=cnt[s],
                            elem_size=D,
                            queue_num=t % gq,
                            single_packet=sp,
                        ).then_inc(g_sems[s], 16)
                    for s in range(NSLOT):
                        gp.wait_ge(g_sems[s], 16 * len(range(s, TOT, NSLOT)))
                elif mode == "scatter":
                    # timing probe: scatter garbage SBUF repeatedly
                    for t in range(TOT):
                        s, k = t % NSLOT, t % NCH
                        base = chunks[k][2]
                        gp.reg_load(cnt[s], counts_t[0:1, k : k + 1])
                        gp.dma_scatter_add(
                            out_ap=out[base : base + WIN, :],
                            in_ap=data[s][:],
                            idxs_ap=sidx_t[:, k * w : (k + 1) * w],
                            num_idxs=CAP,
                            num_idxs_reg=cnt[s],
                            elem_size=D,
                            queue_num=nq - sq + (t % sq),
                            single_packet=sp,
                        ).then_inc(st_sems[s], 16)
                    for s in range(NSLOT):
                        gp.wait_ge(st_sems[s], 16 * len(range(s, TOT, NSLOT)))
                elif mode == "regload":
                    # timing probe: just the reg_loads
                    for t in range(TOT):
                        s, k = t % NSLOT, t % NCH
                        gp.reg_load(cnt[s], counts_t[0:1, k : k + 1])
                    gp.wait_ge(in_sems[0], 16)
                else:
                    raise ValueError(mode)

    nc.compile()
    return nc


def _get_nc(chunks):
    key = ("nc", tuple(chunks))
    if key not in _CACHE:
        _CACHE[key] = _build_nc(chunks)
    return _CACHE[key]


def kernel(input_ids: np.ndarray, table: np.ndarray, **_run_kw) -> np.ndarray:
    chunks, in_maps = prep_inputs(input_ids, table)
    nc = _get_nc(chunks)

    res = run_bass_kernel_spmd(nc, in_maps, list(range(H)), **_run_kw)
    outs = [
        np.asarray(res.results[h]["out"]).reshape(B, L, D) for h in range(H)
    ]
    full = np.stack(outs, axis=2)  # [B, L, H, D]
    if _run_kw:
        _CACHE["last_results"] = res
    return full

